# revision 41
# baseline (speedup 1.0000x reference)
"""Bahdanau additive attention (nn_AttentionModule) on 8 TRN2 NeuronCores.

Math (B=32, S=4096, D=1024, L=1):
    dec[b,e]   = sum_d dhs[0,b,d] * Ua_w[e,d] + Ua_b[e]
    enc[b,s,e] = sum_d eo[b,s,d] * Wa_w[e,d] + Wa_b[e]
    score[b,s] = sum_e Va_w[0,e] * tanh(enc[b,s,e] + dec[b,e])   (+ Va_b, a
                 constant shift that cancels in softmax -> dropped)
    out[b,0,s] = softmax_s(where(mask[b,s], score[b,s], -inf))

Sharding: data-parallel over batch, 4 batches per core; weights replicated.

Masked positions get exactly 0 weight, so only the valid encoder columns are
computed: the host gathers each batch's valid columns (~half of S) and
scatters the results back into a zero-filled output. Batches are sorted by
valid-count and assigned to (core, slot) so each slot's shared capacity (one
SPMD program for all cores) has minimal padding. This is exact.

Device kernel (per core) computes only the RAW scores; the softmax (exp /
normalize, trivially cheap vs the 17 GFLOP of matmul) runs on the host
during the scatter, which removes masks, exp and the normalize multiply from
the device's critical path entirely. Padding columns produce garbage scores
that the host scatter simply never reads.

Precision: eo and Wa are cast to bf16 on the host (end-to-end rel err vs the
fp32 reference ~1.3e-3, well inside the 2e-2 gate). bf16 streams at the same
1 column/cycle PE rate as float32r but halves DMA + SBUF traffic and enables
fast weight load for the per-matmul LDWEIGHTS.

Per-tile structure (tile = up to 512 gathered columns of one batch):
  - enc tiles [e=128, s<=512] accumulated over 8 d-chunks (Wa^T stationary,
    encoder outputs pre-transposed on host to [D, total]).
  - tanh fused with the per-(b,e) bias (dec) on the scalar engine.
  - Va reduction: instead of 8 M=1 matmuls (which cost a full 512-cycle
    column stream each, 12.5% of PE time), the vector engine folds the 8
    tanh chunks into one [128, s] accumulator with fused multiply-add
    (scalar_tensor_tensor: acc = th*va + acc), and the 128-partition reduce
    runs on the otherwise idle GPSIMD engine (partition_all_reduce, ~3.5us,
    hidden under the next tile) with the raw scores DMA'd straight from its
    output row. PE cost drops from 72 to 64 cycles per column -- the
    streaming-rate floor for this algorithm.
  - only the final tile keeps a ones-vector matmul for its reduce (0.3us
    latency instead of 3.5us, shortening the end-of-kernel serial chain);
    slots are ordered so that chain ends on the narrowest tile.

Timing pitfalls this kernel works around (each measured from traces):
  - the PE clock-gate (HAM) starts at 1.2 GHz and only reaches 2.4 GHz
    after ~3.4us of gap-free matmul activity; any early stall re-throttles
    it for tens of us. ~13 warmup matmuls on scratch data (no DMA deps)
    carry the PE through the prologue so real tiles run warm and dense.
  - each dma_start costs ~0.5us of sequencer issue time; the sync queue
    starts issuing ~2.4us before the other engines clear the init barrier,
    so the weight+first-tile loads go there, chunk-interleaved in the order
    the matmuls consume them. va/dec ride the gpsimd queue.
  - device-side transpose DMAs (16B descriptors) take ~15us for even tiny
    tensors -- all layouts are pre-transposed on the host.
  - the activation table load (~1.3us) is triggered by a dummy tanh at
    kernel start so the first real tanh isn't gated on it.
"""

import numpy as np
from contextlib import ExitStack

import ml_dtypes
import concourse.bass as bass
import concourse.tile as tile
from concourse import bacc, bass_isa, mybir
from concourse.bass_utils import run_bass_kernel_spmd

N_CORES = 8
B, S, D = 32, 4096, 1024
B_LOC = B // N_CORES      # 4 batch slots per core
P = 128                   # partitions
D_CH = D // P             # 8 chunks of the contraction/e dims
S_TILE = 512

F32 = mybir.dt.float32
F32R = mybir.dt.float32r
BF16 = mybir.dt.bfloat16
TANH = mybir.ActivationFunctionType.Tanh
MULT = mybir.AluOpType.mult
ADD = mybir.AluOpType.add


def tile_sizes(cap):
    """Split cap into 512-wide tiles; a small remainder is balanced into the
    last two tiles so every matmul keeps N >= 256 (full fp32r rate for the
    ones-reduce, amortized LDWEIGHTS for the bf16 enc matmuls)."""
    k, rem = divmod(cap, S_TILE)
    if rem == 0:
        return [S_TILE] * k
    if k == 0:
        return [rem]
    if rem >= 256:
        return [S_TILE] * k + [rem]
    tot = S_TILE + rem
    s1 = (tot // 2 + 15) // 16 * 16
    return [S_TILE] * (k - 1) + [s1, tot - s1]


def build_bass(caps):
    """caps: per-batch-slot column capacities (same for every core)."""
    slot_sizes = [tile_sizes(c) for c in caps]
    offs = [sum(caps[:i]) for i in range(B_LOC)]
    total = sum(caps)
    nc = bacc.Bacc("TRN2", target_bir_lowering=False, debug=False)

    eoT = nc.dram_tensor("eoT", [D, total], BF16, kind="ExternalInput").ap()
    waT = nc.dram_tensor("waT", [D_CH, P, D], BF16, kind="ExternalInput").ap()
    # vab/decb come pre-transposed from the host (partition-major) -- a
    # device-side transpose DMA decomposes into thousands of 16B descriptors
    # and takes ~15us, gating the first tanh of the whole pipeline.
    vab = nc.dram_tensor("vab", [P, D_CH], F32, kind="ExternalInput").ap()
    decb = nc.dram_tensor("decb", [P, D_CH, B_LOC], F32, kind="ExternalInput").ap()
    out = nc.dram_tensor("out", [1, total], F32, kind="ExternalOutput").ap()

    with tile.TileContext(nc) as tc, ExitStack() as ctx:
        consts = ctx.enter_context(tc.tile_pool(name="consts", bufs=1))
        xpool = ctx.enter_context(tc.tile_pool(name="x", bufs=3))
        tpool = ctx.enter_context(tc.tile_pool(name="tanh", bufs=6))
        apool = ctx.enter_context(tc.tile_pool(name="acc", bufs=3))
        rpool = ctx.enter_context(tc.tile_pool(name="red", bufs=2))
        misc = ctx.enter_context(tc.tile_pool(name="misc", bufs=1))
        ppool = ctx.enter_context(tc.tile_pool(name="enc_psum", bufs=4, space="PSUM"))
        spool = ctx.enter_context(tc.tile_pool(name="score_psum", bufs=3, space="PSUM"))
        wpool = ctx.enter_context(tc.tile_pool(name="warm_psum", bufs=1, space="PSUM"))

        eoT_c = eoT.rearrange("(dc d) s -> d dc s", d=P)

        # Prologue loads FIRST. Each dma_start costs ~0.5us of ISSUE time on
        # its engine's sequencer, and the sync queue starts issuing ~2.4us in
        # (not gated by the ~6.5us engine-init barrier), so the weights and
        # first x tile -- needed first -- go there, interleaved in dc order
        # (the order the matmuls consume them). va/dec ride the gpsimd queue.
        wa_sb = consts.tile([P, D_CH, D], BF16)
        va_sb = consts.tile([P, D_CH], F32)
        dec_sb = consts.tile([P, D_CH, B_LOC], F32)
        nc.gpsimd.dma_start(out=va_sb, in_=vab)
        nc.gpsimd.dma_start(out=dec_sb, in_=decb)
        # Slot processing order: smallest final tile LAST -- the end-of-kernel
        # serial chain (last enc matmul -> tanh -> FMA -> ones-matmul -> copy
        # -> out DMA) scales with the final tile's width.
        slot_order = sorted(range(B_LOC), key=lambda j: -slot_sizes[j][-1])
        first_slot = slot_order[0]
        g00 = offs[first_slot]
        sz0 = slot_sizes[first_slot][0]
        x0 = xpool.tile([P, D_CH, S_TILE], BF16, tag="x_sb")
        for dc in range(D_CH):
            nc.sync.dma_start(out=wa_sb[:, dc, :], in_=waT[dc])
            nc.sync.dma_start(out=x0[:, dc, :sz0], in_=eoT_c[:, dc, g00 : g00 + sz0])

        # ones vector for the final tile's reduce matmul (f32r to match acc;
        # memset can't write f32r directly, hence the bitcast).
        ones_f = consts.tile([P, 1], F32)
        nc.vector.memset(ones_f, 1.0)
        ones = ones_f[:, :].bitcast(F32R)
        # Trigger ACT_TABLE_LOAD (~1.3us) now, while the prologue DMAs are
        # still in flight, so the first real tanh isn't stuck behind it.
        dummy = consts.tile([1, 1], F32)
        nc.vector.memset(dummy, 0.0)
        nc.scalar.activation(out=dummy, in_=dummy, func=TANH)
        # ~4us of warmup matmuls: the HAM clock gate only lifts the PE to
        # 2.4 GHz after a full ~3.4us window of sustained activity, and any
        # gap resets it. Warming keeps the real matmuls at full clock from
        # the start; by the time warmup drains, all prologue DMAs have
        # landed, so tile 0 runs gap-free -- otherwise the whole first tile
        # runs at 1.2 GHz (measured: ~46us instead of ~14us). The warmup
        # reads the first wa chunk (the earliest-landing DMA, ~4us) rather
        # than a memset tile: the memset engines are barrier-gated until
        # ~6.5us, which would delay the PE past its own barrier.
        wps = wpool.tile([1, S_TILE], F32)
        n_warm = 13
        for i in range(n_warm):
            nc.tensor.matmul(
                wps,
                lhsT=wa_sb[:, 0, 0:1],
                rhs=wa_sb[:, 0, 0:S_TILE],
                start=(i == 0),
                stop=(i == n_warm - 1),
            )

        # Staging row for the final tile's scores.
        scores_sb = misc.tile([1, S_TILE], F32)

        # Deferred per-tile reduction: (acc, sz, score slice).
        pend = []

        def flush_pend(final=False):
            if not pend:
                return
            acc, sz, csl = pend.pop(0)
            if final:
                # ones-matmul path: ~0.3us latency, keeps the tail short
                sps = spool.tile([1, S_TILE], F32, tag="sps")
                nc.tensor.matmul(
                    sps[:, :sz], lhsT=ones, rhs=acc[:, :sz], start=True, stop=True
                )
                nc.scalar.copy(out=scores_sb[0:1, :sz], in_=sps[:, :sz])
                nc.sync.dma_start(out=out[0:1, csl], in_=scores_sb[0:1, :sz])
            else:
                # GPSIMD daisy-chain partition reduce: frees ~512 PE cycles
                # per tile; its ~3us latency hides under the next tile.
                red = rpool.tile([P, S_TILE], F32, tag="red")
                nc.gpsimd.partition_all_reduce(
                    out_ap=red[:, :sz],
                    in_ap=acc[:, :sz],
                    channels=P,
                    reduce_op=bass_isa.ReduceOp.add,
                )
                nc.gpsimd.dma_start(out=out[0:1, csl], in_=red[0:1, :sz])

        for b in slot_order:
            sizes = slot_sizes[b]
            for st in range(len(sizes)):
                sz = sizes[st]
                g0 = offs[b] + sum(sizes[:st])
                csl = slice(g0, g0 + sz)
                first = b == first_slot and st == 0
                if first:
                    x_sb = x0
                else:
                    x_sb = xpool.tile([P, D_CH, S_TILE], BF16, tag="x_sb")
                    for dc in range(D_CH):
                        nc.sync.dma_start(
                            out=x_sb[:, dc, :sz], in_=eoT_c[:, dc, g0 : g0 + sz]
                        )
                acc = apool.tile([P, S_TILE], F32R, tag="acc")
                for ec in range(D_CH):
                    eps = ppool.tile([P, S_TILE], F32, tag="eps")
                    for dc in range(D_CH):
                        nc.tensor.matmul(
                            eps[:, :sz],
                            lhsT=wa_sb[:, dc, ec * P : (ec + 1) * P],
                            rhs=x_sb[:, dc, :sz],
                            start=(dc == 0),
                            stop=(dc == D_CH - 1),
                        )
                    th = tpool.tile([P, S_TILE], F32R, tag="th")
                    nc.scalar.activation(
                        out=th[:, :sz],
                        in_=eps[:, :sz],
                        func=TANH,
                        bias=dec_sb[:, ec, b : b + 1],
                        scale=1.0,
                    )
                    if ec == 0:
                        nc.vector.tensor_scalar_mul(
                            out=acc[:, :sz], in0=th[:, :sz], scalar1=va_sb[:, 0:1]
                        )
                    else:
                        nc.vector.scalar_tensor_tensor(
                            out=acc[:, :sz],
                            in0=th[:, :sz],
                            scalar=va_sb[:, ec : ec + 1],
                            in1=acc[:, :sz],
                            op0=MULT,
                            op1=ADD,
                        )
                    if ec == 1:
                        # reduce the PREVIOUS tile here: by now this tile's
                        # first two chunk-groups (~3.4us of matmuls) are ahead
                        # of it in the PE queue, so acc(prev) is long ready.
                        flush_pend()
                pend.append((acc, sz, csl))
        flush_pend(final=True)

    nc.compile()
    return nc


_NC_CACHE = {}


def get_nc(caps):
    if caps not in _NC_CACHE:
        _NC_CACHE[caps] = build_bass(caps)
    return _NC_CACHE[caps]


def prep(
    encoder_outputs, decoder_hidden_state, attn_mask, Wa_w, Wa_b, Ua_w, Ua_b, Va_w, Va_b
):
    """Host-side shard prep.

    Batches are assigned to (core, slot) so that each slot's capacity --
    shared by all cores (one SPMD program) -- is the max valid-count within
    that slot; sorting batches by count first keeps the padding tiny.
    Returns (in_maps, caps, assignment, idxs, counts).
    """
    eo = np.asarray(encoder_outputs, dtype=np.float32)
    dhs = np.asarray(decoder_hidden_state, dtype=np.float32)
    mask = np.asarray(attn_mask).astype(bool)
    wa_w = np.asarray(Wa_w, dtype=np.float32)
    wa_b = np.asarray(Wa_b, dtype=np.float32)
    ua_w = np.asarray(Ua_w, dtype=np.float32)
    ua_b = np.asarray(Ua_b, dtype=np.float32)
    va_w = np.asarray(Va_w, dtype=np.float32)

    idxs = [np.flatnonzero(mask[b]) for b in range(B)]
    counts = [len(ix) for ix in idxs]

    order = sorted(range(B), key=lambda b: -counts[b])
    # assignment[c][j] = original batch index handled by core c, slot j
    assignment = [[order[j * N_CORES + c] for j in range(B_LOC)] for c in range(N_CORES)]
    caps = [
        max(64, ((max(counts[order[j * N_CORES + c]] for c in range(N_CORES)) + 15) // 16) * 16)
        for j in range(B_LOC)
    ]
    offs = [sum(caps[:j]) for j in range(B_LOC)]
    total = sum(caps)

    waT = np.ascontiguousarray(wa_w.T).reshape(D_CH, P, D).astype(ml_dtypes.bfloat16)
    # partition-major [P, D_CH] so the device DMA is contiguous per partition
    vab = np.ascontiguousarray(va_w.reshape(D_CH, P).T)
    # dec[b,e] = Ua @ dhs + Ua_b + Wa_b: tiny per-batch constant, host-folded.
    dec_full = dhs[0] @ ua_w.T + ua_b + wa_b  # [B, D]

    in_maps = []
    for c in range(N_CORES):
        eoT_c = np.zeros((D, total), dtype=ml_dtypes.bfloat16)
        decb_c = np.zeros((P, D_CH, B_LOC), dtype=np.float32)
        for j in range(B_LOC):
            b = assignment[c][j]
            cnt = counts[b]
            eoT_c[:, offs[j] : offs[j] + cnt] = eo[b, idxs[b]].T.astype(
                ml_dtypes.bfloat16
            )
            decb_c[:, :, j] = dec_full[b].reshape(D_CH, P).T
        in_maps.append(
            {
                "eoT": eoT_c,
                "waT": waT,
                "vab": vab,
                "decb": decb_c,
            }
        )
    return in_maps, caps, assignment, idxs, counts


def scatter_out(core_outs, caps, assignment, idxs, counts):
    """Host softmax over each batch's raw scores + scatter to full shape."""
    offs = [sum(caps[:j]) for j in range(B_LOC)]
    w = np.zeros((B, 1, S), dtype=np.float32)
    for c in range(N_CORES):
        row = np.asarray(core_outs[c], dtype=np.float64).reshape(-1)
        for j in range(B_LOC):
            b = assignment[c][j]
            sc = row[offs[j] : offs[j] + counts[b]]
            if len(sc) == 0:
                continue
            e = np.exp(sc - sc.max())
            w[b, 0, idxs[b]] = (e / e.sum()).astype(np.float32)
    return w


def kernel(**inputs) -> np.ndarray:
    in_maps, caps, assignment, idxs, counts = prep(**inputs)
    nc = get_nc(tuple(caps))
    res = run_bass_kernel_spmd(nc, in_maps, list(range(N_CORES)))
    return scatter_out(
        [res.results[i]["out"] for i in range(N_CORES)], caps, assignment, idxs, counts
    )


# revision 42
# speedup vs baseline: 1.0016x; 1.0016x over previous
"""Bahdanau additive attention (nn_AttentionModule) on 8 TRN2 NeuronCores.

Math (B=32, S=4096, D=1024, L=1):
    dec[b,e]   = sum_d dhs[0,b,d] * Ua_w[e,d] + Ua_b[e]
    enc[b,s,e] = sum_d eo[b,s,d] * Wa_w[e,d] + Wa_b[e]
    score[b,s] = sum_e Va_w[0,e] * tanh(enc[b,s,e] + dec[b,e])   (+ Va_b, a
                 constant shift that cancels in softmax -> dropped)
    out[b,0,s] = softmax_s(where(mask[b,s], score[b,s], -inf))

Sharding: data-parallel over batch, 4 batches per core; weights replicated.

Masked positions get exactly 0 weight, so only the valid encoder columns are
computed: the host gathers each batch's valid columns (~half of S) and
scatters the results back into a zero-filled output. Batches are sorted by
valid-count and assigned to (core, slot) so each slot's shared capacity (one
SPMD program for all cores) has minimal padding. This is exact.

Device kernel (per core) computes only the RAW scores; the softmax (exp /
normalize, trivially cheap vs the 17 GFLOP of matmul) runs on the host
during the scatter, which removes masks, exp and the normalize multiply from
the device's critical path entirely. Padding columns produce garbage scores
that the host scatter simply never reads.

Precision: eo and Wa are cast to bf16 on the host (end-to-end rel err vs the
fp32 reference ~1.3e-3, well inside the 2e-2 gate). bf16 streams at the same
1 column/cycle PE rate as float32r but halves DMA + SBUF traffic and enables
fast weight load for the per-matmul LDWEIGHTS.

Per-tile structure (tile = up to 512 gathered columns of one batch):
  - enc tiles [e=128, s<=512] accumulated over 8 d-chunks (Wa^T stationary,
    encoder outputs pre-transposed on host to [D, total]).
  - tanh fused with the per-(b,e) bias (dec) on the scalar engine.
  - Va reduction: instead of 8 M=1 matmuls (which cost a full 512-cycle
    column stream each, 12.5% of PE time), the vector engine folds the 8
    tanh chunks into one [128, s] accumulator with fused multiply-add
    (scalar_tensor_tensor: acc = th*va + acc), and the 128-partition reduce
    runs on the otherwise idle GPSIMD engine (partition_all_reduce, ~3.5us,
    hidden under the next tile) with the raw scores DMA'd straight from its
    output row. PE cost drops from 72 to 64 cycles per column -- the
    streaming-rate floor for this algorithm.
  - only the final tile keeps a ones-vector matmul for its reduce (0.3us
    latency instead of 3.5us, shortening the end-of-kernel serial chain);
    slots are ordered so that chain ends on the narrowest tile.

Timing pitfalls this kernel works around (each measured from traces):
  - the PE clock-gate (HAM) starts at 1.2 GHz and only reaches 2.4 GHz
    after ~3.4us of gap-free matmul activity; any early stall re-throttles
    it for tens of us. ~13 warmup matmuls on scratch data (no DMA deps)
    carry the PE through the prologue so real tiles run warm and dense.
  - each dma_start costs ~0.5us of sequencer issue time; the sync queue
    starts issuing ~2.4us before the other engines clear the init barrier,
    so the weight+first-tile loads go there, chunk-interleaved in the order
    the matmuls consume them. va/dec ride the gpsimd queue.
  - device-side transpose DMAs (16B descriptors) take ~15us for even tiny
    tensors -- all layouts are pre-transposed on the host.
  - the activation table load (~1.3us) is triggered by a dummy tanh at
    kernel start so the first real tanh isn't gated on it.
"""

import numpy as np
from contextlib import ExitStack

import ml_dtypes
import concourse.bass as bass
import concourse.tile as tile
from concourse import bacc, bass_isa, mybir
from concourse.bass_utils import run_bass_kernel_spmd

N_CORES = 8
B, S, D = 32, 4096, 1024
B_LOC = B // N_CORES      # 4 batch slots per core
P = 128                   # partitions
D_CH = D // P             # 8 chunks of the contraction/e dims
S_TILE = 512

F32 = mybir.dt.float32
F32R = mybir.dt.float32r
BF16 = mybir.dt.bfloat16
TANH = mybir.ActivationFunctionType.Tanh
MULT = mybir.AluOpType.mult
ADD = mybir.AluOpType.add


def tile_sizes(cap):
    """Split cap into 512-wide tiles; a small remainder is balanced into the
    last two tiles so every matmul keeps N >= 256 (full fp32r rate for the
    ones-reduce, amortized LDWEIGHTS for the bf16 enc matmuls)."""
    k, rem = divmod(cap, S_TILE)
    if rem == 0:
        return [S_TILE] * k
    if k == 0:
        return [rem]
    if rem >= 256:
        return [S_TILE] * k + [rem]
    tot = S_TILE + rem
    s1 = (tot // 2 + 15) // 16 * 16
    return [S_TILE] * (k - 1) + [s1, tot - s1]


def build_bass(caps):
    """caps: per-batch-slot column capacities (same for every core)."""
    slot_sizes = [tile_sizes(c) for c in caps]
    offs = [sum(caps[:i]) for i in range(B_LOC)]
    total = sum(caps)
    nc = bacc.Bacc("TRN2", target_bir_lowering=False, debug=False)

    eoT = nc.dram_tensor("eoT", [D, total], BF16, kind="ExternalInput").ap()
    waT = nc.dram_tensor("waT", [D_CH, P, D], BF16, kind="ExternalInput").ap()
    # vab/decb come pre-transposed from the host (partition-major) -- a
    # device-side transpose DMA decomposes into thousands of 16B descriptors
    # and takes ~15us, gating the first tanh of the whole pipeline.
    vab = nc.dram_tensor("vab", [P, D_CH], F32, kind="ExternalInput").ap()
    decb = nc.dram_tensor("decb", [P, D_CH, B_LOC], F32, kind="ExternalInput").ap()
    out = nc.dram_tensor("out", [1, total], F32, kind="ExternalOutput").ap()

    with tile.TileContext(nc) as tc, ExitStack() as ctx:
        consts = ctx.enter_context(tc.tile_pool(name="consts", bufs=1))
        xpool = ctx.enter_context(tc.tile_pool(name="x", bufs=3))
        tpool = ctx.enter_context(tc.tile_pool(name="tanh", bufs=6))
        apool = ctx.enter_context(tc.tile_pool(name="acc", bufs=3))
        rpool = ctx.enter_context(tc.tile_pool(name="red", bufs=2))
        misc = ctx.enter_context(tc.tile_pool(name="misc", bufs=1))
        ppool = ctx.enter_context(tc.tile_pool(name="enc_psum", bufs=4, space="PSUM"))
        spool = ctx.enter_context(tc.tile_pool(name="score_psum", bufs=3, space="PSUM"))
        wpool = ctx.enter_context(tc.tile_pool(name="warm_psum", bufs=1, space="PSUM"))

        eoT_c = eoT.rearrange("(dc d) s -> d dc s", d=P)

        # Prologue loads FIRST. Each dma_start costs ~0.5us of ISSUE time on
        # its engine's sequencer, and the sync queue starts issuing ~2.4us in
        # (not gated by the ~6.5us engine-init barrier), so the weights and
        # first x tile -- needed first -- go there, interleaved in dc order
        # (the order the matmuls consume them). va/dec ride the gpsimd queue.
        wa_sb = consts.tile([P, D_CH, D], BF16)
        va_sb = consts.tile([P, D_CH], F32)
        dec_sb = consts.tile([P, D_CH, B_LOC], F32)
        nc.gpsimd.dma_start(out=va_sb, in_=vab)
        nc.gpsimd.dma_start(out=dec_sb, in_=decb)
        # Slot processing order: smallest final tile LAST -- the end-of-kernel
        # serial chain (last enc matmul -> tanh -> FMA -> ones-matmul -> copy
        # -> out DMA) scales with the final tile's width.
        slot_order = sorted(range(B_LOC), key=lambda j: -slot_sizes[j][-1])
        first_slot = slot_order[0]
        g00 = offs[first_slot]
        sz0 = slot_sizes[first_slot][0]
        x0 = xpool.tile([P, D_CH, S_TILE], BF16, tag="x_sb")
        for dc in range(D_CH):
            nc.sync.dma_start(out=wa_sb[:, dc, :], in_=waT[dc])
            nc.sync.dma_start(out=x0[:, dc, :sz0], in_=eoT_c[:, dc, g00 : g00 + sz0])

        # ones vector for the final tile's reduce matmul plus a zeroed
        # warmup operand (f32r for the PE via bitcast; memset can't write
        # f32r directly). Memsets run as soon as the engine-init barrier
        # clears (~6.5us) -- earlier than any 256KB prologue DMA can land
        # (a single dma_start rides one ~27GiB/s queue, ~9us for a wa
        # chunk), so warmup on memset data starts the PE soonest.
        ones_f = consts.tile([P, 1], F32)
        scratch_f = consts.tile([P, S_TILE], F32)
        nc.vector.memset(ones_f, 1.0)
        nc.vector.memset(scratch_f, 0.0)
        ones = ones_f[:, :].bitcast(F32R)
        scratch = scratch_f[:, :].bitcast(F32R)
        # Trigger ACT_TABLE_LOAD (~1.3us) now, while the prologue DMAs are
        # still in flight, so the first real tanh isn't stuck behind it.
        dummy = consts.tile([1, 1], F32)
        nc.vector.memset(dummy, 0.0)
        nc.scalar.activation(out=dummy, in_=dummy, func=TANH)
        # ~5.5us of warmup matmuls: the HAM clock gate only lifts the PE to
        # 2.4 GHz after a full ~3.4us window of sustained activity, and any
        # gap resets it. Warming on scratch data (no DMA deps) keeps the
        # real matmuls at full clock from the start; by the time warmup
        # drains, the prologue DMAs have landed, so tile 0 runs gap-free --
        # otherwise the whole first tile runs at 1.2 GHz (measured: ~46us
        # instead of ~14us).
        wps = wpool.tile([1, S_TILE], F32)
        n_warm = 13
        for i in range(n_warm):
            nc.tensor.matmul(
                wps, lhsT=ones, rhs=scratch, start=(i == 0), stop=(i == n_warm - 1)
            )

        # Staging row for the final tile's scores.
        scores_sb = misc.tile([1, S_TILE], F32)

        # Deferred per-tile reduction: (acc, sz, score slice).
        pend = []

        def flush_pend(final=False):
            if not pend:
                return
            acc, sz, csl = pend.pop(0)
            if final:
                # ones-matmul path: ~0.3us latency, keeps the tail short
                sps = spool.tile([1, S_TILE], F32, tag="sps")
                nc.tensor.matmul(
                    sps[:, :sz], lhsT=ones, rhs=acc[:, :sz], start=True, stop=True
                )
                nc.scalar.copy(out=scores_sb[0:1, :sz], in_=sps[:, :sz])
                nc.sync.dma_start(out=out[0:1, csl], in_=scores_sb[0:1, :sz])
            else:
                # GPSIMD daisy-chain partition reduce: frees ~512 PE cycles
                # per tile; its ~3us latency hides under the next tile.
                red = rpool.tile([P, S_TILE], F32, tag="red")
                nc.gpsimd.partition_all_reduce(
                    out_ap=red[:, :sz],
                    in_ap=acc[:, :sz],
                    channels=P,
                    reduce_op=bass_isa.ReduceOp.add,
                )
                nc.gpsimd.dma_start(out=out[0:1, csl], in_=red[0:1, :sz])

        for b in slot_order:
            sizes = slot_sizes[b]
            for st in range(len(sizes)):
                sz = sizes[st]
                g0 = offs[b] + sum(sizes[:st])
                csl = slice(g0, g0 + sz)
                first = b == first_slot and st == 0
                if first:
                    x_sb = x0
                else:
                    x_sb = xpool.tile([P, D_CH, S_TILE], BF16, tag="x_sb")
                    for dc in range(D_CH):
                        nc.sync.dma_start(
                            out=x_sb[:, dc, :sz], in_=eoT_c[:, dc, g0 : g0 + sz]
                        )
                acc = apool.tile([P, S_TILE], F32R, tag="acc")
                for ec in range(D_CH):
                    eps = ppool.tile([P, S_TILE], F32, tag="eps")
                    for dc in range(D_CH):
                        nc.tensor.matmul(
                            eps[:, :sz],
                            lhsT=wa_sb[:, dc, ec * P : (ec + 1) * P],
                            rhs=x_sb[:, dc, :sz],
                            start=(dc == 0),
                            stop=(dc == D_CH - 1),
                        )
                    th = tpool.tile([P, S_TILE], F32R, tag="th")
                    nc.scalar.activation(
                        out=th[:, :sz],
                        in_=eps[:, :sz],
                        func=TANH,
                        bias=dec_sb[:, ec, b : b + 1],
                        scale=1.0,
                    )
                    if ec == 0:
                        nc.vector.tensor_scalar_mul(
                            out=acc[:, :sz], in0=th[:, :sz], scalar1=va_sb[:, 0:1]
                        )
                    else:
                        nc.vector.scalar_tensor_tensor(
                            out=acc[:, :sz],
                            in0=th[:, :sz],
                            scalar=va_sb[:, ec : ec + 1],
                            in1=acc[:, :sz],
                            op0=MULT,
                            op1=ADD,
                        )
                    if ec == 1:
                        # reduce the PREVIOUS tile here: by now this tile's
                        # first two chunk-groups (~3.4us of matmuls) are ahead
                        # of it in the PE queue, so acc(prev) is long ready.
                        flush_pend()
                pend.append((acc, sz, csl))
        flush_pend(final=True)

    nc.compile()
    return nc


_NC_CACHE = {}


def get_nc(caps):
    if caps not in _NC_CACHE:
        _NC_CACHE[caps] = build_bass(caps)
    return _NC_CACHE[caps]


def prep(
    encoder_outputs, decoder_hidden_state, attn_mask, Wa_w, Wa_b, Ua_w, Ua_b, Va_w, Va_b
):
    """Host-side shard prep.

    Batches are assigned to (core, slot) so that each slot's capacity --
    shared by all cores (one SPMD program) -- is the max valid-count within
    that slot; sorting batches by count first keeps the padding tiny.
    Returns (in_maps, caps, assignment, idxs, counts).
    """
    eo = np.asarray(encoder_outputs, dtype=np.float32)
    dhs = np.asarray(decoder_hidden_state, dtype=np.float32)
    mask = np.asarray(attn_mask).astype(bool)
    wa_w = np.asarray(Wa_w, dtype=np.float32)
    wa_b = np.asarray(Wa_b, dtype=np.float32)
    ua_w = np.asarray(Ua_w, dtype=np.float32)
    ua_b = np.asarray(Ua_b, dtype=np.float32)
    va_w = np.asarray(Va_w, dtype=np.float32)

    idxs = [np.flatnonzero(mask[b]) for b in range(B)]
    counts = [len(ix) for ix in idxs]

    order = sorted(range(B), key=lambda b: -counts[b])
    # assignment[c][j] = original batch index handled by core c, slot j
    assignment = [[order[j * N_CORES + c] for j in range(B_LOC)] for c in range(N_CORES)]
    caps = [
        max(64, ((max(counts[order[j * N_CORES + c]] for c in range(N_CORES)) + 15) // 16) * 16)
        for j in range(B_LOC)
    ]
    offs = [sum(caps[:j]) for j in range(B_LOC)]
    total = sum(caps)

    waT = np.ascontiguousarray(wa_w.T).reshape(D_CH, P, D).astype(ml_dtypes.bfloat16)
    # partition-major [P, D_CH] so the device DMA is contiguous per partition
    vab = np.ascontiguousarray(va_w.reshape(D_CH, P).T)
    # dec[b,e] = Ua @ dhs + Ua_b + Wa_b: tiny per-batch constant, host-folded.
    dec_full = dhs[0] @ ua_w.T + ua_b + wa_b  # [B, D]

    in_maps = []
    for c in range(N_CORES):
        eoT_c = np.zeros((D, total), dtype=ml_dtypes.bfloat16)
        decb_c = np.zeros((P, D_CH, B_LOC), dtype=np.float32)
        for j in range(B_LOC):
            b = assignment[c][j]
            cnt = counts[b]
            eoT_c[:, offs[j] : offs[j] + cnt] = eo[b, idxs[b]].T.astype(
                ml_dtypes.bfloat16
            )
            decb_c[:, :, j] = dec_full[b].reshape(D_CH, P).T
        in_maps.append(
            {
                "eoT": eoT_c,
                "waT": waT,
                "vab": vab,
                "decb": decb_c,
            }
        )
    return in_maps, caps, assignment, idxs, counts


def scatter_out(core_outs, caps, assignment, idxs, counts):
    """Host softmax over each batch's raw scores + scatter to full shape."""
    offs = [sum(caps[:j]) for j in range(B_LOC)]
    w = np.zeros((B, 1, S), dtype=np.float32)
    for c in range(N_CORES):
        row = np.asarray(core_outs[c], dtype=np.float64).reshape(-1)
        for j in range(B_LOC):
            b = assignment[c][j]
            sc = row[offs[j] : offs[j] + counts[b]]
            if len(sc) == 0:
                continue
            e = np.exp(sc - sc.max())
            w[b, 0, idxs[b]] = (e / e.sum()).astype(np.float32)
    return w


def kernel(**inputs) -> np.ndarray:
    in_maps, caps, assignment, idxs, counts = prep(**inputs)
    nc = get_nc(tuple(caps))
    res = run_bass_kernel_spmd(nc, in_maps, list(range(N_CORES)))
    return scatter_out(
        [res.results[i]["out"] for i in range(N_CORES)], caps, assignment, idxs, counts
    )


# revision 43
# speedup vs baseline: 1.0021x; 1.0005x over previous
"""Bahdanau additive attention (nn_AttentionModule) on 8 TRN2 NeuronCores.

Math (B=32, S=4096, D=1024, L=1):
    dec[b,e]   = sum_d dhs[0,b,d] * Ua_w[e,d] + Ua_b[e]
    enc[b,s,e] = sum_d eo[b,s,d] * Wa_w[e,d] + Wa_b[e]
    score[b,s] = sum_e Va_w[0,e] * tanh(enc[b,s,e] + dec[b,e])   (+ Va_b, a
                 constant shift that cancels in softmax -> dropped)
    out[b,0,s] = softmax_s(where(mask[b,s], score[b,s], -inf))

Sharding: data-parallel over batch, 4 batches per core; weights replicated.

Masked positions get exactly 0 weight, so only the valid encoder columns are
computed: the host gathers each batch's valid columns (~half of S) and
scatters the results back into a zero-filled output. Batches are sorted by
valid-count and assigned to (core, slot) so each slot's shared capacity (one
SPMD program for all cores) has minimal padding. This is exact.

Device kernel (per core) computes only the RAW scores; the softmax (exp /
normalize, trivially cheap vs the 17 GFLOP of matmul) runs on the host
during the scatter, which removes masks, exp and the normalize multiply from
the device's critical path entirely. Padding columns produce garbage scores
that the host scatter simply never reads.

Precision: eo and Wa are cast to bf16 on the host (end-to-end rel err vs the
fp32 reference ~1.3e-3, well inside the 2e-2 gate). bf16 streams at the same
1 column/cycle PE rate as float32r but halves DMA + SBUF traffic and enables
fast weight load for the per-matmul LDWEIGHTS.

Per-tile structure (tile = up to 512 gathered columns of one batch):
  - enc tiles [e=128, s<=512] accumulated over 8 d-chunks (Wa^T stationary,
    encoder outputs pre-transposed on host to [D, total]).
  - tanh fused with the per-(b,e) bias (dec) on the scalar engine.
  - Va reduction: instead of 8 M=1 matmuls (which cost a full 512-cycle
    column stream each, 12.5% of PE time), the vector engine folds the 8
    tanh chunks into one [128, s] accumulator with fused multiply-add
    (scalar_tensor_tensor: acc = th*va + acc), and the 128-partition reduce
    runs on the otherwise idle GPSIMD engine (partition_all_reduce, ~3.5us,
    hidden under the next tile) with the raw scores DMA'd straight from its
    output row. PE cost drops from 72 to 64 cycles per column -- the
    streaming-rate floor for this algorithm.
  - only the final tile keeps a ones-vector matmul for its reduce (0.3us
    latency instead of 3.5us, shortening the end-of-kernel serial chain);
    slots are ordered so that chain ends on the narrowest tile.

Timing pitfalls this kernel works around (each measured from traces):
  - the PE clock-gate (HAM) starts at 1.2 GHz and only reaches 2.4 GHz
    after ~3.4us of gap-free matmul activity; any early stall re-throttles
    it for tens of us. ~13 warmup matmuls on scratch data (no DMA deps)
    carry the PE through the prologue so real tiles run warm and dense.
  - each dma_start costs ~0.5us of sequencer issue time; the sync queue
    starts issuing ~2.4us before the other engines clear the init barrier,
    so the weight+first-tile loads go there, chunk-interleaved in the order
    the matmuls consume them. va/dec ride the gpsimd queue.
  - device-side transpose DMAs (16B descriptors) take ~15us for even tiny
    tensors -- all layouts are pre-transposed on the host.
  - the activation table load (~1.3us) is triggered by a dummy tanh at
    kernel start so the first real tanh isn't gated on it.
"""

import numpy as np
from contextlib import ExitStack

import ml_dtypes
import concourse.bass as bass
import concourse.tile as tile
from concourse import bacc, bass_isa, mybir
from concourse.bass_utils import run_bass_kernel_spmd

N_CORES = 8
B, S, D = 32, 4096, 1024
B_LOC = B // N_CORES      # 4 batch slots per core
P = 128                   # partitions
D_CH = D // P             # 8 chunks of the contraction/e dims
S_TILE = 512

F32 = mybir.dt.float32
F32R = mybir.dt.float32r
BF16 = mybir.dt.bfloat16
TANH = mybir.ActivationFunctionType.Tanh
MULT = mybir.AluOpType.mult
ADD = mybir.AluOpType.add


def tile_sizes(cap):
    """Split cap into 512-wide tiles; a small remainder is balanced into the
    last two tiles so every matmul keeps N >= 256 (full fp32r rate for the
    ones-reduce, amortized LDWEIGHTS for the bf16 enc matmuls)."""
    k, rem = divmod(cap, S_TILE)
    if rem == 0:
        return [S_TILE] * k
    if k == 0:
        return [rem]
    if rem >= 256:
        return [S_TILE] * k + [rem]
    tot = S_TILE + rem
    s1 = (tot // 2 + 15) // 16 * 16
    return [S_TILE] * (k - 1) + [s1, tot - s1]


def build_bass(caps):
    """caps: per-batch-slot column capacities (same for every core)."""
    slot_sizes = [tile_sizes(c) for c in caps]
    offs = [sum(caps[:i]) for i in range(B_LOC)]
    total = sum(caps)
    nc = bacc.Bacc("TRN2", target_bir_lowering=False, debug=False)

    eoT = nc.dram_tensor("eoT", [D, total], BF16, kind="ExternalInput").ap()
    waT = nc.dram_tensor("waT", [D_CH, P, D], BF16, kind="ExternalInput").ap()
    # vab/decb come pre-transposed from the host (partition-major) -- a
    # device-side transpose DMA decomposes into thousands of 16B descriptors
    # and takes ~15us, gating the first tanh of the whole pipeline.
    vab = nc.dram_tensor("vab", [P, D_CH], F32, kind="ExternalInput").ap()
    decb = nc.dram_tensor("decb", [P, D_CH, B_LOC], F32, kind="ExternalInput").ap()
    out = nc.dram_tensor("out", [1, total], F32, kind="ExternalOutput").ap()

    with tile.TileContext(nc) as tc, ExitStack() as ctx:
        consts = ctx.enter_context(tc.tile_pool(name="consts", bufs=1))
        xpool = ctx.enter_context(tc.tile_pool(name="x", bufs=3))
        tpool = ctx.enter_context(tc.tile_pool(name="tanh", bufs=6))
        apool = ctx.enter_context(tc.tile_pool(name="acc", bufs=3))
        rpool = ctx.enter_context(tc.tile_pool(name="red", bufs=2))
        misc = ctx.enter_context(tc.tile_pool(name="misc", bufs=1))
        ppool = ctx.enter_context(tc.tile_pool(name="enc_psum", bufs=4, space="PSUM"))
        spool = ctx.enter_context(tc.tile_pool(name="score_psum", bufs=3, space="PSUM"))
        wpool = ctx.enter_context(tc.tile_pool(name="warm_psum", bufs=1, space="PSUM"))

        eoT_c = eoT.rearrange("(dc d) s -> d dc s", d=P)

        # Prologue loads FIRST. Each dma_start costs ~0.5us of ISSUE time on
        # its engine's sequencer, and the sync queue starts issuing ~2.4us in
        # (not gated by the ~6.5us engine-init barrier), so the weights and
        # first x tile -- needed first -- go there, interleaved in dc order
        # (the order the matmuls consume them). va/dec ride the gpsimd queue.
        wa_sb = consts.tile([P, D_CH, D], BF16)
        va_sb = consts.tile([P, D_CH], F32)
        dec_sb = consts.tile([P, D_CH, B_LOC], F32)
        nc.gpsimd.dma_start(out=va_sb, in_=vab)
        nc.gpsimd.dma_start(out=dec_sb, in_=decb)
        # Slot processing order: smallest final tile LAST -- the end-of-kernel
        # serial chain (last enc matmul -> tanh -> FMA -> ones-matmul -> copy
        # -> out DMA) scales with the final tile's width.
        slot_order = sorted(range(B_LOC), key=lambda j: -slot_sizes[j][-1])
        first_slot = slot_order[0]
        g00 = offs[first_slot]
        sz0 = slot_sizes[first_slot][0]
        x0 = xpool.tile([P, D_CH, S_TILE], BF16, tag="x_sb")
        # A single dma_start rides ONE ~27GiB/s queue, so a full 256KB wa
        # chunk takes ~9us to land. Each chunk is split in half (an enc
        # matmul's 128-col lhsT slice lies entirely in one half, so per-MM
        # waits stay at wa+x = 2 sems): the ec<4 halves and x0 land first,
        # in consumption order; the ec>=4 halves follow.
        H = D // 2
        for dc in range(D_CH):
            nc.sync.dma_start(out=wa_sb[:, dc, 0:H], in_=waT[dc, :, 0:H])
            nc.sync.dma_start(out=x0[:, dc, :sz0], in_=eoT_c[:, dc, g00 : g00 + sz0])
        for dc in range(D_CH):
            nc.sync.dma_start(out=wa_sb[:, dc, H:D], in_=waT[dc, :, H:D])

        # ones vector for the final tile's reduce matmul plus a zeroed
        # warmup operand (f32r for the PE via bitcast; memset can't write
        # f32r directly). Memsets run as soon as the engine-init barrier
        # clears (~6.5us) -- earlier than any 256KB prologue DMA can land
        # (a single dma_start rides one ~27GiB/s queue, ~9us for a wa
        # chunk), so warmup on memset data starts the PE soonest.
        ones_f = consts.tile([P, 1], F32)
        scratch_f = consts.tile([P, S_TILE], F32)
        nc.vector.memset(ones_f, 1.0)
        nc.vector.memset(scratch_f, 0.0)
        ones = ones_f[:, :].bitcast(F32R)
        scratch = scratch_f[:, :].bitcast(F32R)
        # Trigger ACT_TABLE_LOAD (~1.3us) now, while the prologue DMAs are
        # still in flight, so the first real tanh isn't stuck behind it.
        dummy = consts.tile([1, 1], F32)
        nc.vector.memset(dummy, 0.0)
        nc.scalar.activation(out=dummy, in_=dummy, func=TANH)
        # ~5.5us of warmup matmuls: the HAM clock gate only lifts the PE to
        # 2.4 GHz after a full ~3.4us window of sustained activity, and any
        # gap resets it. Warming on scratch data (no DMA deps) keeps the
        # real matmuls at full clock from the start; by the time warmup
        # drains, the prologue DMAs have landed, so tile 0 runs gap-free --
        # otherwise the whole first tile runs at 1.2 GHz (measured: ~46us
        # instead of ~14us).
        wps = wpool.tile([1, S_TILE], F32)
        n_warm = 13
        for i in range(n_warm):
            nc.tensor.matmul(
                wps, lhsT=ones, rhs=scratch, start=(i == 0), stop=(i == n_warm - 1)
            )

        # Staging row for the final tile's scores.
        scores_sb = misc.tile([1, S_TILE], F32)

        # Deferred per-tile reduction: (acc, sz, score slice).
        pend = []

        def flush_pend(final=False):
            if not pend:
                return
            acc, sz, csl = pend.pop(0)
            if final:
                # ones-matmul path: ~0.3us latency, keeps the tail short
                sps = spool.tile([1, S_TILE], F32, tag="sps")
                nc.tensor.matmul(
                    sps[:, :sz], lhsT=ones, rhs=acc[:, :sz], start=True, stop=True
                )
                nc.scalar.copy(out=scores_sb[0:1, :sz], in_=sps[:, :sz])
                nc.sync.dma_start(out=out[0:1, csl], in_=scores_sb[0:1, :sz])
            else:
                # GPSIMD daisy-chain partition reduce: frees ~512 PE cycles
                # per tile; its ~3us latency hides under the next tile.
                red = rpool.tile([P, S_TILE], F32, tag="red")
                nc.gpsimd.partition_all_reduce(
                    out_ap=red[:, :sz],
                    in_ap=acc[:, :sz],
                    channels=P,
                    reduce_op=bass_isa.ReduceOp.add,
                )
                nc.gpsimd.dma_start(out=out[0:1, csl], in_=red[0:1, :sz])

        for b in slot_order:
            sizes = slot_sizes[b]
            for st in range(len(sizes)):
                sz = sizes[st]
                g0 = offs[b] + sum(sizes[:st])
                csl = slice(g0, g0 + sz)
                first = b == first_slot and st == 0
                if first:
                    x_sb = x0
                else:
                    x_sb = xpool.tile([P, D_CH, S_TILE], BF16, tag="x_sb")
                    for dc in range(D_CH):
                        nc.sync.dma_start(
                            out=x_sb[:, dc, :sz], in_=eoT_c[:, dc, g0 : g0 + sz]
                        )
                acc = apool.tile([P, S_TILE], F32R, tag="acc")
                for ec in range(D_CH):
                    eps = ppool.tile([P, S_TILE], F32, tag="eps")
                    for dc in range(D_CH):
                        nc.tensor.matmul(
                            eps[:, :sz],
                            lhsT=wa_sb[:, dc, ec * P : (ec + 1) * P],
                            rhs=x_sb[:, dc, :sz],
                            start=(dc == 0),
                            stop=(dc == D_CH - 1),
                        )
                    th = tpool.tile([P, S_TILE], F32R, tag="th")
                    nc.scalar.activation(
                        out=th[:, :sz],
                        in_=eps[:, :sz],
                        func=TANH,
                        bias=dec_sb[:, ec, b : b + 1],
                        scale=1.0,
                    )
                    if ec == 0:
                        nc.vector.tensor_scalar_mul(
                            out=acc[:, :sz], in0=th[:, :sz], scalar1=va_sb[:, 0:1]
                        )
                    else:
                        nc.vector.scalar_tensor_tensor(
                            out=acc[:, :sz],
                            in0=th[:, :sz],
                            scalar=va_sb[:, ec : ec + 1],
                            in1=acc[:, :sz],
                            op0=MULT,
                            op1=ADD,
                        )
                    if ec == 1:
                        # reduce the PREVIOUS tile here: by now this tile's
                        # first two chunk-groups (~3.4us of matmuls) are ahead
                        # of it in the PE queue, so acc(prev) is long ready.
                        flush_pend()
                pend.append((acc, sz, csl))
        flush_pend(final=True)

    nc.compile()
    return nc


_NC_CACHE = {}


def get_nc(caps):
    if caps not in _NC_CACHE:
        _NC_CACHE[caps] = build_bass(caps)
    return _NC_CACHE[caps]


def prep(
    encoder_outputs, decoder_hidden_state, attn_mask, Wa_w, Wa_b, Ua_w, Ua_b, Va_w, Va_b
):
    """Host-side shard prep.

    Batches are assigned to (core, slot) so that each slot's capacity --
    shared by all cores (one SPMD program) -- is the max valid-count within
    that slot; sorting batches by count first keeps the padding tiny.
    Returns (in_maps, caps, assignment, idxs, counts).
    """
    eo = np.asarray(encoder_outputs, dtype=np.float32)
    dhs = np.asarray(decoder_hidden_state, dtype=np.float32)
    mask = np.asarray(attn_mask).astype(bool)
    wa_w = np.asarray(Wa_w, dtype=np.float32)
    wa_b = np.asarray(Wa_b, dtype=np.float32)
    ua_w = np.asarray(Ua_w, dtype=np.float32)
    ua_b = np.asarray(Ua_b, dtype=np.float32)
    va_w = np.asarray(Va_w, dtype=np.float32)

    idxs = [np.flatnonzero(mask[b]) for b in range(B)]
    counts = [len(ix) for ix in idxs]

    order = sorted(range(B), key=lambda b: -counts[b])
    # assignment[c][j] = original batch index handled by core c, slot j
    assignment = [[order[j * N_CORES + c] for j in range(B_LOC)] for c in range(N_CORES)]
    caps = [
        max(64, ((max(counts[order[j * N_CORES + c]] for c in range(N_CORES)) + 15) // 16) * 16)
        for j in range(B_LOC)
    ]
    offs = [sum(caps[:j]) for j in range(B_LOC)]
    total = sum(caps)

    waT = np.ascontiguousarray(wa_w.T).reshape(D_CH, P, D).astype(ml_dtypes.bfloat16)
    # partition-major [P, D_CH] so the device DMA is contiguous per partition
    vab = np.ascontiguousarray(va_w.reshape(D_CH, P).T)
    # dec[b,e] = Ua @ dhs + Ua_b + Wa_b: tiny per-batch constant, host-folded.
    dec_full = dhs[0] @ ua_w.T + ua_b + wa_b  # [B, D]

    in_maps = []
    for c in range(N_CORES):
        eoT_c = np.zeros((D, total), dtype=ml_dtypes.bfloat16)
        decb_c = np.zeros((P, D_CH, B_LOC), dtype=np.float32)
        for j in range(B_LOC):
            b = assignment[c][j]
            cnt = counts[b]
            eoT_c[:, offs[j] : offs[j] + cnt] = eo[b, idxs[b]].T.astype(
                ml_dtypes.bfloat16
            )
            decb_c[:, :, j] = dec_full[b].reshape(D_CH, P).T
        in_maps.append(
            {
                "eoT": eoT_c,
                "waT": waT,
                "vab": vab,
                "decb": decb_c,
            }
        )
    return in_maps, caps, assignment, idxs, counts


def scatter_out(core_outs, caps, assignment, idxs, counts):
    """Host softmax over each batch's raw scores + scatter to full shape."""
    offs = [sum(caps[:j]) for j in range(B_LOC)]
    w = np.zeros((B, 1, S), dtype=np.float32)
    for c in range(N_CORES):
        row = np.asarray(core_outs[c], dtype=np.float64).reshape(-1)
        for j in range(B_LOC):
            b = assignment[c][j]
            sc = row[offs[j] : offs[j] + counts[b]]
            if len(sc) == 0:
                continue
            e = np.exp(sc - sc.max())
            w[b, 0, idxs[b]] = (e / e.sum()).astype(np.float32)
    return w


def kernel(**inputs) -> np.ndarray:
    in_maps, caps, assignment, idxs, counts = prep(**inputs)
    nc = get_nc(tuple(caps))
    res = run_bass_kernel_spmd(nc, in_maps, list(range(N_CORES)))
    return scatter_out(
        [res.results[i]["out"] for i in range(N_CORES)], caps, assignment, idxs, counts
    )
